# revision 2
# baseline (speedup 1.0000x reference)
"""Trainium2 Bass kernel for CustomizeL2Loss (vm-major DVE/ACT design).

Reference (x, y: (N, C, T, V, M) = (256, 3, 600, 25, 2) f32):
    motion    = x[:, :, 1:] - x[:, :, :-1]
    mean_move = mean(|motion|, axis=(C, T-1, M))            -> (N, V)
    ratio     = V * mean_move / sum_v(mean_move)            -> (N, V)
    loss      = mean((x - y)**2 * ratio[:, None, None, :, None])

By linearity only S[n,v] = sum (x-y)^2 and A[n,v] = sum |motion| over
(c, t, m) are needed; the (256, 25) ratio/loss math runs on the host in f64.

Device layout (data-parallel, 8 cores x 32 samples):
  partition p = 4*n_local + q where q is a quarter of T (150 t's). The host
  pretransposes to vm-major x_d [128, VM, C, 152] / y_d [128, VM, C, 150]
  f32, t innermost: x t-slot 1 is the previous quarter's last t so motion
  pairs never cross partitions (for q=0 it duplicates t=0, zeroing that
  pair); slot 0 is dead so the x-vs-y sub reads even-aligned slices
  (DVE 2x-packing). Chunks slice VM, so every chunk holds the full (c,t)
  extent: the SWDGE cast-DMA (fp32 HBM -> bf16 SBUF) reads 604B contiguous
  runs at HBM line rate (the kernel is DMA-bound at ~23 MB/core). DVE does
  the two subs and the motion (c,t)-reduce with abs fused in; the (x-y)^2
  sum runs on ACT as per-vm Square activations with fused fp32 accum_out
  (one activation-table entry, no sq intermediate). One tiny output DMA.
  Host folds q/m partials and does the (256, 25) ratio/loss math in f64.
"""

from contextlib import ExitStack

import numpy as np

import concourse.bass as bass
import concourse.bacc as bacc
import concourse.tile as tile
from concourse import mybir
from concourse.bass_utils import run_bass_kernel_spmd

N, C, T, V, M = 256, 3, 600, 25, 2
N_CORES = 8
NL = N // N_CORES        # 32 samples per core
Q = 4                    # t-quarters -> 128 partitions
TQ = T // Q              # 150 t's per partition
VM = V * M               # 50
CHUNKS = (12, 12, 12, 10, 4)  # vm-chunk sizes; small tail chunk shortens
assert sum(CHUNKS) == VM      # the trailing DMA->compute dependency chain

F32 = mybir.dt.float32
BF16 = mybir.dt.bfloat16
FP16 = mybir.dt.float16

AX_XY = mybir.AxisListType.XY
ADD = mybir.AluOpType.add


def build_program(reps: int = 1, io_bufs: int = 3, wk_bufs: int = 2,
                  out_dt=F32, chunks=CHUNKS, act_accum: bool = True):
    nc = bacc.Bacc("TRN2", target_bir_lowering=False, debug=False)
    x_d = nc.dram_tensor("xp", [128, VM, C, TQ + 2], F32, kind="ExternalInput").ap()
    y_d = nc.dram_tensor("yp", [128, VM, C, TQ], F32, kind="ExternalInput").ap()
    out_d = nc.dram_tensor("out", [128, 2, VM], out_dt, kind="ExternalOutput").ap()

    with tile.TileContext(nc) as tc, ExitStack() as ctx:
        io = ctx.enter_context(tc.tile_pool(name="io", bufs=io_bufs))
        work = ctx.enter_context(tc.tile_pool(name="work", bufs=wk_bufs))
        sas = ctx.enter_context(tc.tile_pool(name="sas", bufs=2))

        with nc.allow_low_precision(reason="fp16 partials of ~450-elem sums"):
            for _ in range(reps):
                sa = sas.tile([128, 2, VM], out_dt, tag="sa")
                v0 = 0
                mvc = max(chunks)
                for vc in chunks:
                    xt_f = io.tile([128, mvc, C, TQ + 2], BF16, tag="x")
                    yt_f = io.tile([128, mvc, C, TQ], BF16, tag="y")
                    d_f = work.tile([128, mvc, C, TQ], BF16, tag="d")
                    d1_f = work.tile([128, mvc, C, TQ], BF16, tag="d1")
                    xt, yt = xt_f[:, 0:vc], yt_f[:, 0:vc]
                    d, d1 = d_f[:, 0:vc], d1_f[:, 0:vc]
                    nc.gpsimd.dma_start(out=xt, in_=x_d[:, v0 : v0 + vc])
                    nc.gpsimd.dma_start(out=yt, in_=y_d[:, v0 : v0 + vc])
                    # x data lives at t-slots 2..151 so both operands of the
                    # d sub are even-aligned (2x-packable); slot 1 holds the
                    # previous quarter's last t for the motion pair.
                    nc.vector.tensor_sub(d, xt[:, :, :, 2:], yt)
                    nc.vector.tensor_sub(d1, xt[:, :, :, 1 : TQ + 1], xt[:, :, :, 2:])
                    if act_accum:
                        for v in range(vc):
                            sc_f = work.tile([128, 1, C, TQ], BF16, tag="sc")
                            nc.scalar.activation(
                                sc_f, d[:, v : v + 1],
                                mybir.ActivationFunctionType.Square,
                                accum_out=sa[:, 0, v0 + v : v0 + v + 1],
                            )
                    else:
                        sq_f = work.tile([128, mvc, C, TQ], BF16, tag="sq")
                        sq = sq_f[:, 0:vc]
                        nc.scalar.activation(
                            sq, d, mybir.ActivationFunctionType.Square)
                        nc.vector.tensor_reduce(
                            sa[:, 0, v0 : v0 + vc], sq, AX_XY, ADD)
                    nc.vector.tensor_reduce(
                        sa[:, 1, v0 : v0 + vc], d1, AX_XY, ADD,
                        apply_absolute_value=True,
                    )
                    v0 += vc
                nc.sync.dma_start(out=out_d, in_=sa)

    nc.compile()
    return nc


def make_in_maps(x, y):
    # (N, C, T, V, M) -> per core vm-major x_d [128, VM, C, TQ+1] (padded),
    # y_d [128, VM, C, TQ]; p = 4*n_local + q.
    def to_vm_major(a):
        a = a.reshape(N_CORES, NL, C, Q, TQ, VM)
        return a.transpose(0, 1, 3, 5, 2, 4)  # (cores, n, q, vm, c, t)

    xr = to_vm_major(x)
    yr = np.ascontiguousarray(to_vm_major(y)).reshape(N_CORES, 128, VM, C, TQ)
    xp = np.empty((N_CORES, NL, Q, VM, C, TQ + 2), np.float32)
    xp[..., 2:] = xr
    xp[:, :, 1:, :, :, 1] = xr[:, :, :-1, :, :, -1]  # prev quarter's last t
    xp[:, :, 0, :, :, 1] = xr[:, :, 0, :, :, 0]      # q=0: duplicate t=0
    xp[..., 0] = 0.0                                 # dead alignment slot
    xp = xp.reshape(N_CORES, 128, VM, C, TQ + 2)
    return [{"xp": xp[k], "yp": yr[k]} for k in range(N_CORES)]


def host_reduce(outs: np.ndarray) -> np.float32:
    """outs: (N_CORES, 128, 2, VM) -> scalar loss."""
    o = outs.astype(np.float64)
    o = o.reshape(N_CORES, NL, Q, 2, V, M).sum(axis=(2, 5))  # fold q, m
    S = o[:, :, 0].reshape(N, V)
    A = o[:, :, 1].reshape(N, V)
    num = (A * S).sum(axis=1)
    den = A.sum(axis=1)
    loss = (V * num / den).sum() / float(N * C * T * V * M)
    return np.float32(loss)


_NC_CACHE = None


def kernel(x: np.ndarray, y: np.ndarray) -> np.ndarray:
    global _NC_CACHE
    x = np.asarray(x, dtype=np.float32)
    y = np.asarray(y, dtype=np.float32)
    in_maps = make_in_maps(x, y)
    if _NC_CACHE is None:
        _NC_CACHE = build_program()
    res = run_bass_kernel_spmd(_NC_CACHE, in_maps, list(range(N_CORES))).results
    outs = np.stack([res[k]["out"] for k in range(N_CORES)])
    return host_reduce(outs)


# revision 3
# speedup vs baseline: 1.0907x; 1.0907x over previous
"""Trainium2 Bass kernel for CustomizeL2Loss (vm-major DVE/ACT design).

Reference (x, y: (N, C, T, V, M) = (256, 3, 600, 25, 2) f32):
    motion    = x[:, :, 1:] - x[:, :, :-1]
    mean_move = mean(|motion|, axis=(C, T-1, M))            -> (N, V)
    ratio     = V * mean_move / sum_v(mean_move)            -> (N, V)
    loss      = mean((x - y)**2 * ratio[:, None, None, :, None])

By linearity only S[n,v] = sum (x-y)^2 and A[n,v] = sum |motion| over
(c, t, m) are needed; the (256, 25) ratio/loss math runs on the host in f64.

Device layout (data-parallel, 8 cores x 32 samples):
  partition p = 4*n_local + q where q is a quarter of T (150 t's). The host
  pretransposes to vm-major x_d [128, VM, C, 152] / y_d [128, VM, C, 150]
  f32, t innermost: x t-slot 1 is the previous quarter's last t so motion
  pairs never cross partitions (for q=0 it duplicates t=0, zeroing that
  pair); slot 0 is dead so the x-vs-y sub reads even-aligned slices
  (DVE 2x-packing). Chunks slice VM, so every chunk holds the full (c,t)
  extent: the SWDGE cast-DMA (fp32 HBM -> bf16 SBUF) reads 604B contiguous
  runs at HBM line rate (the kernel is DMA-bound at ~23 MB/core). DVE does
  the two subs and the motion (c,t)-reduce with abs fused in; the (x-y)^2
  sum runs on ACT as per-vm Square activations with fused fp32 accum_out
  (one activation-table entry, no sq intermediate). One tiny output DMA.
  Host folds q/m partials and does the (256, 25) ratio/loss math in f64.
"""

from contextlib import ExitStack

import numpy as np

import concourse.bass as bass
import concourse.bacc as bacc
import concourse.tile as tile
from concourse import mybir
from concourse.bass_utils import run_bass_kernel_spmd

N, C, T, V, M = 256, 3, 600, 25, 2
N_CORES = 8
NL = N // N_CORES        # 32 samples per core
Q = 4                    # t-quarters -> 128 partitions
TQ = T // Q              # 150 t's per partition
VM = V * M               # 50
CHUNKS = (10, 10, 10, 10, 10)  # uniform vm-chunks measured fastest
assert sum(CHUNKS) == VM

F32 = mybir.dt.float32
BF16 = mybir.dt.bfloat16
FP16 = mybir.dt.float16

AX_XY = mybir.AxisListType.XY
ADD = mybir.AluOpType.add


def build_program(reps: int = 1, io_bufs: int = 4, wk_bufs: int = 2,
                  out_dt=F32, chunks=CHUNKS, act_accum: bool = True):
    nc = bacc.Bacc("TRN2", target_bir_lowering=False, debug=False)
    x_d = nc.dram_tensor("xp", [128, VM, C, TQ + 2], F32, kind="ExternalInput").ap()
    y_d = nc.dram_tensor("yp", [128, VM, C, TQ], F32, kind="ExternalInput").ap()
    out_d = nc.dram_tensor("out", [128, 2, VM], out_dt, kind="ExternalOutput").ap()

    with tile.TileContext(nc) as tc, ExitStack() as ctx:
        io = ctx.enter_context(tc.tile_pool(name="io", bufs=io_bufs))
        work = ctx.enter_context(tc.tile_pool(name="work", bufs=wk_bufs))
        sas = ctx.enter_context(tc.tile_pool(name="sas", bufs=2))

        with nc.allow_low_precision(reason="fp16 partials of ~450-elem sums"):
            for _ in range(reps):
                sa = sas.tile([128, 2, VM], out_dt, tag="sa")
                v0 = 0
                mvc = max(chunks)
                for vc in chunks:
                    xt_f = io.tile([128, mvc, C, TQ + 2], BF16, tag="x")
                    yt_f = io.tile([128, mvc, C, TQ], BF16, tag="y")
                    d_f = work.tile([128, mvc, C, TQ], BF16, tag="d")
                    d1_f = work.tile([128, mvc, C, TQ], BF16, tag="d1")
                    xt, yt = xt_f[:, 0:vc], yt_f[:, 0:vc]
                    d, d1 = d_f[:, 0:vc], d1_f[:, 0:vc]
                    nc.gpsimd.dma_start(out=xt, in_=x_d[:, v0 : v0 + vc])
                    nc.gpsimd.dma_start(out=yt, in_=y_d[:, v0 : v0 + vc])
                    # x data lives at t-slots 2..151 so both operands of the
                    # d sub are even-aligned (2x-packable); slot 1 holds the
                    # previous quarter's last t for the motion pair.
                    nc.vector.tensor_sub(d, xt[:, :, :, 2:], yt)
                    nc.vector.tensor_sub(d1, xt[:, :, :, 1 : TQ + 1], xt[:, :, :, 2:])
                    if act_accum:
                        for v in range(vc):
                            sc_f = work.tile([128, 1, C, TQ], BF16, tag="sc")
                            nc.scalar.activation(
                                sc_f, d[:, v : v + 1],
                                mybir.ActivationFunctionType.Square,
                                accum_out=sa[:, 0, v0 + v : v0 + v + 1],
                            )
                    else:
                        sq_f = work.tile([128, mvc, C, TQ], BF16, tag="sq")
                        sq = sq_f[:, 0:vc]
                        nc.scalar.activation(
                            sq, d, mybir.ActivationFunctionType.Square)
                        nc.vector.tensor_reduce(
                            sa[:, 0, v0 : v0 + vc], sq, AX_XY, ADD)
                    nc.vector.tensor_reduce(
                        sa[:, 1, v0 : v0 + vc], d1, AX_XY, ADD,
                        apply_absolute_value=True,
                    )
                    v0 += vc
                nc.sync.dma_start(out=out_d, in_=sa)

    nc.compile()
    return nc


def make_in_maps(x, y):
    # (N, C, T, V, M) -> per core vm-major x_d [128, VM, C, TQ+1] (padded),
    # y_d [128, VM, C, TQ]; p = 4*n_local + q.
    def to_vm_major(a):
        a = a.reshape(N_CORES, NL, C, Q, TQ, VM)
        return a.transpose(0, 1, 3, 5, 2, 4)  # (cores, n, q, vm, c, t)

    xr = to_vm_major(x)
    yr = np.ascontiguousarray(to_vm_major(y)).reshape(N_CORES, 128, VM, C, TQ)
    xp = np.empty((N_CORES, NL, Q, VM, C, TQ + 2), np.float32)
    xp[..., 2:] = xr
    xp[:, :, 1:, :, :, 1] = xr[:, :, :-1, :, :, -1]  # prev quarter's last t
    xp[:, :, 0, :, :, 1] = xr[:, :, 0, :, :, 0]      # q=0: duplicate t=0
    xp[..., 0] = 0.0                                 # dead alignment slot
    xp = xp.reshape(N_CORES, 128, VM, C, TQ + 2)
    return [{"xp": xp[k], "yp": yr[k]} for k in range(N_CORES)]


def host_reduce(outs: np.ndarray) -> np.float32:
    """outs: (N_CORES, 128, 2, VM) -> scalar loss."""
    o = outs.astype(np.float64)
    o = o.reshape(N_CORES, NL, Q, 2, V, M).sum(axis=(2, 5))  # fold q, m
    S = o[:, :, 0].reshape(N, V)
    A = o[:, :, 1].reshape(N, V)
    num = (A * S).sum(axis=1)
    den = A.sum(axis=1)
    loss = (V * num / den).sum() / float(N * C * T * V * M)
    return np.float32(loss)


_NC_CACHE = None


def kernel(x: np.ndarray, y: np.ndarray) -> np.ndarray:
    global _NC_CACHE
    x = np.asarray(x, dtype=np.float32)
    y = np.asarray(y, dtype=np.float32)
    in_maps = make_in_maps(x, y)
    if _NC_CACHE is None:
        _NC_CACHE = build_program()
    res = run_bass_kernel_spmd(_NC_CACHE, in_maps, list(range(N_CORES))).results
    outs = np.stack([res[k]["out"] for k in range(N_CORES)])
    return host_reduce(outs)


# revision 4
# speedup vs baseline: 1.1877x; 1.0890x over previous
"""Trainium2 Bass kernel for CustomizeL2Loss — v2 (vm-major DVE design).

Reference (x, y: (N, C, T, V, M) = (256, 3, 600, 25, 2) f32):
    motion    = x[:, :, 1:] - x[:, :, :-1]
    mean_move = mean(|motion|, axis=(C, T-1, M))            -> (N, V)
    ratio     = V * mean_move / sum_v(mean_move)            -> (N, V)
    loss      = mean((x - y)**2 * ratio[:, None, None, :, None])

By linearity only S[n,v] = sum (x-y)^2 and A[n,v] = sum |motion| over
(c, t, m) are needed; the (256, 25) ratio/loss math runs on the host in f64.

Device layout (data-parallel, 8 cores x 32 samples):
  partition p = 4*n_local + q where q is a quarter of T (150 t's). The host
  pretransposes to vm-major x_d [128, VM, C, 151] / y_d [128, VM, C, 150]
  f32 (t innermost; x row t=0 is the previous quarter's last t so motion
  pairs never cross partitions; for q=0 it duplicates t=0 making that pair
  exactly 0). Chunks slice VM (5 x 10), so every chunk holds the full (c,t)
  extent: the SWDGE cast-DMA (fp32 HBM -> bf16 SBUF) reads 604B contiguous
  runs, and on DVE both tensor_subs and both (c,t) tensor_reduces (abs
  fused into the motion reduce) run in 2x-packed mode with unit-stride t.
  ACT runs only Square (one activation table entry). Reduce outputs are
  fp16 (DVE reduce accumulates wider internally; partials are ~450-element
  sums), folded on the host. One tiny output DMA per rep.
"""

from contextlib import ExitStack

import numpy as np

import concourse.bass as bass
import concourse.bacc as bacc
import concourse.tile as tile
from concourse import mybir
from concourse.bass_utils import run_bass_kernel_spmd

N, C, T, V, M = 256, 3, 600, 25, 2
N_CORES = 8
NL = N // N_CORES        # 32 samples per core
Q = 4                    # t-quarters -> 128 partitions
TQ = T // Q              # 150 t's per partition
VM = V * M               # 50
CHUNKS = (12, 12, 12, 10, 4)  # vm-chunk sizes; small tail chunk shortens
assert sum(CHUNKS) == VM      # the trailing DMA->compute dependency chain

F32 = mybir.dt.float32
BF16 = mybir.dt.bfloat16
FP16 = mybir.dt.float16

AX_XY = mybir.AxisListType.XY
ADD = mybir.AluOpType.add


def build_program(reps: int = 1, io_bufs: int = 4, wk_bufs: int = 2,
                  out_dt=F32, chunks=CHUNKS, act_accum: bool = True):
    W = TQ + 2 + TQ  # per-(vm,c) run: [pad, pad, x t0..t149, y t0..t149]
    nc = bacc.Bacc("TRN2", target_bir_lowering=False, debug=False)
    xy_d = nc.dram_tensor("xyp", [128, VM, C, W], F32, kind="ExternalInput").ap()
    out_d = nc.dram_tensor("out", [128, 2, VM], out_dt, kind="ExternalOutput").ap()

    with tile.TileContext(nc) as tc, ExitStack() as ctx:
        io = ctx.enter_context(tc.tile_pool(name="io", bufs=io_bufs))
        work = ctx.enter_context(tc.tile_pool(name="work", bufs=wk_bufs))
        sas = ctx.enter_context(tc.tile_pool(name="sas", bufs=2))

        with nc.allow_low_precision(reason="fp16 partials of ~450-elem sums"):
            for _ in range(reps):
                sa = sas.tile([128, 2, VM], out_dt, tag="sa")
                v0 = 0
                mvc = max(chunks)
                for vc in chunks:
                    xy_f = io.tile([128, mvc, C, W], BF16, tag="xy")
                    d_f = work.tile([128, mvc, C, TQ], BF16, tag="d")
                    d1_f = work.tile([128, mvc, C, TQ], BF16, tag="d1")
                    xyt = xy_f[:, 0:vc]
                    d, d1 = d_f[:, 0:vc], d1_f[:, 0:vc]
                    nc.gpsimd.dma_start(out=xyt, in_=xy_d[:, v0 : v0 + vc])
                    # x data at t-slots 2..151 (slot 1 = previous quarter's
                    # last t for the motion pair, slot 0 dead so the d sub
                    # reads even-aligned slices); y data at slots 152..301.
                    xt = xyt[:, :, :, 0 : TQ + 2]
                    yt = xyt[:, :, :, TQ + 2 : W]
                    nc.vector.tensor_sub(d, xt[:, :, :, 2:], yt)
                    nc.vector.tensor_sub(d1, xt[:, :, :, 1 : TQ + 1], xt[:, :, :, 2:])
                    if act_accum:
                        for v in range(vc):
                            sc_f = work.tile([128, 1, C, TQ], BF16, tag="sc")
                            nc.scalar.activation(
                                sc_f, d[:, v : v + 1],
                                mybir.ActivationFunctionType.Square,
                                accum_out=sa[:, 0, v0 + v : v0 + v + 1],
                            )
                    else:
                        sq_f = work.tile([128, mvc, C, TQ], BF16, tag="sq")
                        sq = sq_f[:, 0:vc]
                        nc.scalar.activation(
                            sq, d, mybir.ActivationFunctionType.Square)
                        nc.vector.tensor_reduce(
                            sa[:, 0, v0 : v0 + vc], sq, AX_XY, ADD)
                    nc.vector.tensor_reduce(
                        sa[:, 1, v0 : v0 + vc], d1, AX_XY, ADD,
                        apply_absolute_value=True,
                    )
                    v0 += vc
                nc.sync.dma_start(out=out_d, in_=sa)

    nc.compile()
    return nc


def make_in_maps(x, y):
    # (N, C, T, V, M) -> per core vm-major x_d [128, VM, C, TQ+1] (padded),
    # y_d [128, VM, C, TQ]; p = 4*n_local + q.
    def to_vm_major(a):
        a = a.reshape(N_CORES, NL, C, Q, TQ, VM)
        return a.transpose(0, 1, 3, 5, 2, 4)  # (cores, n, q, vm, c, t)

    xr = to_vm_major(x)
    yr = np.ascontiguousarray(to_vm_major(y)).reshape(N_CORES, 128, VM, C, TQ)
    xp = np.empty((N_CORES, NL, Q, VM, C, TQ + 2), np.float32)
    xp[..., 2:] = xr
    xp[:, :, 1:, :, :, 1] = xr[:, :, :-1, :, :, -1]  # prev quarter's last t
    xp[:, :, 0, :, :, 1] = xr[:, :, 0, :, :, 0]      # q=0: duplicate t=0
    xp[..., 0] = 0.0                                 # dead alignment slot
    xp = xp.reshape(N_CORES, 128, VM, C, TQ + 2)
    xy = np.concatenate([xp, yr], axis=-1)           # one fused run per (vm, c)
    return [{"xyp": xy[k]} for k in range(N_CORES)]


def host_reduce(outs: np.ndarray) -> np.float32:
    """outs: (N_CORES, 128, 2, VM) -> scalar loss."""
    o = outs.astype(np.float64)
    o = o.reshape(N_CORES, NL, Q, 2, V, M).sum(axis=(2, 5))  # fold q, m
    S = o[:, :, 0].reshape(N, V)
    A = o[:, :, 1].reshape(N, V)
    num = (A * S).sum(axis=1)
    den = A.sum(axis=1)
    loss = (V * num / den).sum() / float(N * C * T * V * M)
    return np.float32(loss)


_NC_CACHE = None


def kernel(x: np.ndarray, y: np.ndarray) -> np.ndarray:
    global _NC_CACHE
    x = np.asarray(x, dtype=np.float32)
    y = np.asarray(y, dtype=np.float32)
    in_maps = make_in_maps(x, y)
    if _NC_CACHE is None:
        _NC_CACHE = build_program()
    res = run_bass_kernel_spmd(_NC_CACHE, in_maps, list(range(N_CORES))).results
    outs = np.stack([res[k]["out"] for k in range(N_CORES)])
    return host_reduce(outs)
